# revision 41
# baseline (speedup 1.0000x reference)
"""Causal self-attention (ALiBi + QK-RMSNorm + subln) on 8 TRN2 NeuronCores.

Sharding: 8 cores = 2 batches x 4 head-groups (4 heads / 128 head-dim each).
Per core: QKV projection for its 512 features, attention for its 4 heads,
and a partial output projection (row slice of Wproj); host sums the 4
partials per batch.

v4 design:
- q/k/v stay SBUF-resident; no DRAM round trip (phase A = two passes over x:
  k+v, then q). wk/wq are host-packed so ft-major tiles are contiguous DMAs.
- The softmax denominator is never computed: subln (RMSNorm over head_dim)
  is scale-invariant per (t, head), so per-column factors - the softmax
  denominator and the per-column ALiBi term - cancel. All heads use exact
  per-j-tile biases: exp arg = rk*s + slope*(dj - 128*m) via the activation
  scale/bias operands (m = 4*ic - jt, one bias column per (h, m)); the two
  steep heads (h<2) additionally add the exact per-column ramp
  (slope*dj - slope*col) on the vector engine so unnormalized y stays in
  fp32 range. j-tiles whose entire contribution underflows are skipped.
- Attention is software-pipelined: QK(jt+1) is emitted before AV(jt) so the
  in-order tensor queue never stalls on the exp of tile jt.
- subln rstd = rsqrt(sum y^2) via Newton iteration on the vector engine (no
  scalar Sqrt -> no activation-table thrash with Exp); the sqrt(128) and
  subln_w factors are folded into Wproj host-side.

All matmuls run as float32r (full-rate fp32 streaming with ~11-bit input
rounding, fp32 accumulation).
"""
import math

import numpy as np

import concourse.bacc as bacc
import concourse.bass as bass
import concourse.mybir as mybir
from concourse.tile import TileContext

F32 = mybir.dt.float32
F32R = mybir.dt.float32r
I32 = mybir.dt.int32
AF = mybir.ActivationFunctionType
ALU = mybir.AluOpType

B, T, C = 2, 2048, 2048
H, D = 16, 128
HG = 4          # head groups = cores per batch
HPG = 4         # heads per group
F = HPG * D     # 512 per-core qkv features
EPS = 1e-5
NEG = -1.0e30
TC = 4          # 512-wide t-chunks
NCT = 16        # 128-wide contraction tiles over C
MAGIC = 0x5F3759DF

# j-tile skip thresholds (worst case over the 4 groups' slopes per in-group
# head index): tiles with m = 4*ic - jt > M_THR contribute < e^-85 relative.
M_THR = {0: 1, 1: 4, 2: 99, 3: 99}

# packed-constants layout (columns of cpack [128, CPK])
CPK_MASK = 0          # 128: causal mask for the diagonal block
CPK_WQK = 128         # 1: q_rms_w * k_rms_w
CPK_RAMP = 129        # 2*512: per-column ramp slope*(dj - col) for h=0,1
CPK_MB = 1153         # 2*16: -128*slope*m for h=0,1 (m+3 indexed)
CPK_BT = 1185         # 2*16: slope*(dj - 128*m) for h=2,3 (m+3 indexed)
CPK = 1217


def _alibi_slopes(n_heads):
    def pow2(n):
        start = 2 ** (-(2 ** (-(math.log2(n) - 3))))
        return [start * start**i for i in range(n)]

    if math.log2(n_heads).is_integer():
        return pow2(n_heads)
    c = 2 ** math.floor(math.log2(n_heads))
    s = pow2(c)
    extra = _alibi_slopes(2 * c)
    return s + extra[0::2][: n_heads - c]


def _build():
    nc = bacc.Bacc("TRN2", target_bir_lowering=False)

    xt = nc.dram_tensor("xt", [C, T], F32R, kind="ExternalInput")
    # wq/wk host-packed: [128, ft*ct*128] so ft-major tiles are contiguous
    wq = nc.dram_tensor("wq", [128, 4 * NCT * 128], F32R, kind="ExternalInput")
    wk = nc.dram_tensor("wk", [128, 4 * NCT * 128], F32R, kind="ExternalInput")
    wv = nc.dram_tensor("wv", [C, F], F32R, kind="ExternalInput")
    wp = nc.dram_tensor("wp", [F, C], F32R, kind="ExternalInput")
    cpack = nc.dram_tensor("cpack", [128, CPK], F32, kind="ExternalInput")
    out = nc.dram_tensor("out", [T, C], F32, kind="ExternalOutput")

    xt_r = xt.rearrange("(ct p) t -> p ct t", p=128)
    wq_r = wq.rearrange("p (ft ct d) -> p ft ct d", ft=4, ct=NCT, d=128)
    wk_r = wk.rearrange("p (ft ct d) -> p ft ct d", ft=4, ct=NCT, d=128)
    wv_r = wv.rearrange("(ct p) f -> p ct f", p=128)
    wp_r = wp.rearrange("(ht p) c -> p ht c", p=128)

    with nc.allow_low_precision(reason="f32r rounding of matmul operands is intentional"), TileContext(nc) as tc:
        with (
            tc.tile_pool(name="consts", bufs=1) as consts,
            tc.tile_pool(name="kv", bufs=1) as kv,
        ):
            cpk = consts.tile([128, CPK], F32, tag="cpk")
            nc.sync.dma_start(out=cpk, in_=cpack[:, :])
            mask_t = cpk[:, CPK_MASK:CPK_MASK + 128]
            wqk_t = cpk[:, CPK_WQK:CPK_WQK + 1]

            onesc_t = consts.tile([128, 1], F32R, tag="onesc_t")
            nc.vector.memset(onesc_t.bitcast(F32), 1.0)
            onesr_t = consts.tile([1, 128], F32R, tag="onesr_t")
            nc.vector.memset(onesr_t.bitcast(F32), 1.0)
            eps_c = consts.tile([128, 1], F32, tag="eps_c")
            nc.vector.memset(eps_c, EPS)
            eps128_r = consts.tile([1, 1], F32, tag="eps128_r")
            nc.vector.memset(eps128_r, 128.0 * EPS)
            magic_i = consts.tile([1, 512], I32, tag="magic_i")
            nc.vector.memset(magic_i, MAGIC)
            zeros_r = consts.tile([128, 384], F32R, tag="zeros_r")
            nc.vector.memset(zeros_r.bitcast(F32), 0.0)
            ones_m = consts.tile([128, 256], F32R, tag="ones_m")
            nc.vector.memset(ones_m.bitcast(F32), 1.0)

            # rk columns (1/rms of k) for all (head, j-tile)
            rk_all = consts.tile([128, HPG * 16], F32, tag="rk_all")

            # SBUF-resident k-hat (per head, [d, t]) and v (per t-block, [t, f])
            k_sb = [kv.tile([128, T], F32R, tag=f"k_sb{h}", name=f"k_sb{h}")
                    for h in range(HPG)]
            v_sb = [kv.tile([128, F], F32R, tag=f"v_sb{j}", name=f"v_sb{j}")
                    for j in range(16)]

            # ---------------- Phase A pass 1: k + v ----------------------
            with (
                tc.tile_pool(name="w1", bufs=1) as w1,
                tc.tile_pool(name="xp1", bufs=1) as xp1,
                tc.tile_pool(name="stg1", bufs=1) as stg1,
                tc.tile_pool(name="psum1", bufs=1, space="PSUM") as psum,
            ):
                wk_f = []
                for ftg in range(4):
                    t_ = w1.tile([128, NCT, 128], F32R, tag=f"wk{ftg}",
                                 name=f"wk{ftg}")
                    nc.sync.dma_start(out=t_, in_=wk_r[:, ftg])
                    wk_f.append(t_)
                wv_g = []
                for g in range(4):
                    t_ = w1.tile([128, 4, 512], F32R, tag=f"wv{g}", name=f"wv{g}")
                    nc.sync.dma_start(out=t_, in_=wv_r[:, 4 * g:4 * g + 4, :])
                    wv_g.append(t_)

                # keep the PE busy while the first DMAs land so HAM
                # un-throttles before the real chains start
                warm_ps = psum.tile([1, 256], F32, tag="warm", bufs=1,
                                    name="warm_ps")
                for _ in range(30):
                    nc.tensor.matmul(warm_ps, onesc_t, ones_m,
                                     start=True, stop=True)

                pending_aux = []
                for tch in range(TC):
                    xg = []
                    for s in range(4):
                        t_ = xp1.tile([128, 4, 512], F32R, tag="x1", bufs=6,
                                      name="x1t")
                        nc.scalar.dma_start(
                            out=t_,
                            in_=xt_r[:, 4 * s:4 * s + 4,
                                     tch * 512:(tch + 1) * 512],
                        )
                        xg.append(t_)

                    ksq = []
                    for ft in range(4):
                        ps = psum.tile([128, 512], F32, tag="big_ps", bufs=3)
                        for ct in range(NCT):
                            nc.tensor.matmul(
                                ps,
                                wk_f[ft][:, ct, :],
                                xg[ct // 4][:, ct % 4, :],
                                start=(ct == 0),
                                stop=(ct == NCT - 1),
                            )
                        st = stg1.tile([128, 512], F32R, tag="st", bufs=2)
                        nc.scalar.copy(st, ps)
                        kq = stg1.tile([128, 512], F32R, tag=f"ksq{ft}", bufs=2)
                        nc.vector.tensor_mul(kq, st.bitcast(F32), st.bitcast(F32))
                        ksq.append(kq)
                        nc.vector.tensor_scalar_mul(
                            k_sb[ft][:, tch * 512:(tch + 1) * 512],
                            st.bitcast(F32), scalar1=wqk_t,
                        )
                    for ts4 in range(4):
                        ps = psum.tile([128, 512], F32, tag="big_ps", bufs=3)
                        for ct in range(NCT):
                            nc.tensor.matmul(
                                ps,
                                xg[ct // 4][:, ct % 4, ts4 * 128:(ts4 + 1) * 128],
                                wv_g[ct // 4][:, ct % 4, :],
                                start=(ct == 0),
                                stop=(ct == NCT - 1),
                            )
                        nc.scalar.copy(v_sb[tch * 4 + ts4], ps)

                    # k-norm aux for this tch, emitted one tch later so the
                    # small matmuls never stall the tensor queue.
                    def k_aux(tch=tch, ksq=ksq):
                        for ft in range(4):
                            for ts4 in range(4):
                                jt = tch * 4 + ts4
                                psk = psum.tile([128, 256], F32, tag="psk",
                                                bufs=2)
                                nc.tensor.matmul(
                                    psk, ksq[ft][:, ts4 * 128:(ts4 + 1) * 128],
                                    ones_m, start=True, stop=True,
                                )
                                col = rk_all[:, ft * 16 + jt:ft * 16 + jt + 1]
                                nc.scalar.activation(
                                    col, psk[:, 0:1], AF.Sqrt,
                                    scale=1.0 / 128.0, bias=eps_c,
                                )
                                nc.vector.reciprocal(col, col)

                    if pending_aux:
                        pending_aux.pop()()
                    pending_aux.append(k_aux)
                for fn in pending_aux:
                    fn()

            # ---------------- Phase A pass 2: q -------------------------
            with tc.tile_pool(name="qpool", bufs=1) as qpool:
                q_sb = [qpool.tile([128, T], F32R, tag=f"q_sb{h}", name=f"q_sb{h}")
                        for h in range(HPG)]
                with (
                    tc.tile_pool(name="w2", bufs=1) as w2,
                    tc.tile_pool(name="xp2", bufs=1) as xp2,
                    tc.tile_pool(name="stg2", bufs=1) as stg2,
                    tc.tile_pool(name="psum2", bufs=1, space="PSUM") as psum,
                ):
                    wq_f = []
                    for ftg in range(4):
                        t_ = w2.tile([128, NCT, 128], F32R, tag=f"wq{ftg}",
                                     name=f"wq{ftg}")
                        nc.sync.dma_start(out=t_, in_=wq_r[:, ftg])
                        wq_f.append(t_)

                    pending_aux = []
                    for tch in range(TC):
                        xg = []
                        for s in range(4):
                            t_ = xp2.tile([128, 4, 512], F32R, tag="x2", bufs=6,
                                          name="x2t")
                            nc.scalar.dma_start(
                                out=t_,
                                in_=xt_r[:, 4 * s:4 * s + 4,
                                         tch * 512:(tch + 1) * 512],
                            )
                            xg.append(t_)

                        for ft in range(4):
                            ps = psum.tile([128, 512], F32, tag="big_ps", bufs=3)
                            for ct in range(NCT):
                                nc.tensor.matmul(
                                    ps,
                                    wq_f[ft][:, ct, :],
                                    xg[ct // 4][:, ct % 4, :],
                                    start=(ct == 0),
                                    stop=(ct == NCT - 1),
                                )
                            st = stg2.tile([128, 512], F32R, tag="st", bufs=4)
                            nc.scalar.copy(st, ps)
                            qsq = stg2.tile([128, 512], F32R, tag="qsq", bufs=2)
                            nc.vector.tensor_mul(
                                qsq, st.bitcast(F32), st.bitcast(F32)
                            )

                            def q_aux(tch=tch, ft=ft, st=st, qsq=qsq):
                                ps_row = psum.tile([1, 512], F32, tag="ps_row",
                                                   bufs=2)
                                nc.tensor.matmul(ps_row, onesc_t, qsq,
                                                 start=True, stop=True)
                                rq_f = stg2.tile([1, 512], F32, tag="rq_f")
                                nc.scalar.activation(
                                    rq_f, ps_row, AF.Sqrt, scale=1.0,
                                    bias=eps128_r,
                                )
                                rq_f2 = stg2.tile([1, 512], F32, tag="rq_f2")
                                nc.vector.reciprocal_approx_fast(rq_f2, rq_f)
                                rq_row = stg2.tile([1, 512], F32R, tag="rq_row")
                                nc.vector.tensor_copy(rq_row, rq_f2)
                                ps_b = psum.tile([128, 512], F32, tag="ps_b",
                                                 bufs=2)
                                nc.tensor.matmul(ps_b, onesr_t, rq_row,
                                                 start=True, stop=True)
                                nc.vector.tensor_mul(
                                    q_sb[ft][:, tch * 512:(tch + 1) * 512],
                                    st.bitcast(F32), ps_b,
                                )

                            if pending_aux:
                                pending_aux.pop()()
                            pending_aux.append(q_aux)
                    for fn in pending_aux:
                        fn()

                # ---------------- Phase B: attention per head -----------
                with (
                    tc.tile_pool(name="head", bufs=1) as head,
                    tc.tile_pool(name="ppool", bufs=3) as ppool,
                    tc.tile_pool(name="yfin_pool", bufs=1) as yfin_pool,
                    tc.tile_pool(name="small", bufs=2) as small,
                ):
                    wp_t = head.tile([128, HPG, C], F32R, tag="wp_t", bufs=1)
                    nc.sync.dma_start(out=wp_t, in_=wp_r)

                    # yfin[0] is fresh; heads 1-3 reuse k_sb[h-1], which is
                    # fully consumed before head h writes it.
                    yfin = [yfin_pool.tile([128, T], F32R, tag="yfin0",
                                           name="yfin0")]
                    for h in range(1, HPG):
                        yfin.append(k_sb[h - 1])

                    ysum_sb = []  # [1,512] per (h, ic) for deferred subln rstd
                    for idx in range(HPG * TC):
                        yt = small.tile([1, 512], F32R, tag=f"ysum{idx}", bufs=1,
                                        name=f"ysum{idx}")
                        ysum_sb.append(yt)

                    def rsqrt_inplace(m_row):
                        # m_row <- rsqrt(m_row), Newton iteration on the
                        # vector engine (no scalar Sqrt -> no table thrash)
                        half = small.tile([1, 512], I32, tag="nw_h", bufs=1)
                        nc.vector.tensor_scalar(
                            half, m_row.bitcast(I32), scalar1=1, scalar2=None,
                            op0=ALU.logical_shift_right,
                        )
                        r = small.tile([1, 512], F32, tag="nw_r", bufs=2)
                        nc.vector.tensor_sub(r.bitcast(I32), magic_i, half)
                        for it in range(3):
                            u = small.tile([1, 512], F32, tag="nw_t", bufs=1)
                            nc.vector.tensor_mul(u, r, r)
                            v = small.tile([1, 512], F32, tag="nw_v", bufs=1)
                            nc.vector.tensor_mul(v, m_row.bitcast(F32), u)
                            w = small.tile([1, 512], F32, tag="nw_t", bufs=1)
                            nc.vector.tensor_scalar(
                                w, v, scalar1=-0.5, scalar2=1.5,
                                op0=ALU.mult, op1=ALU.add,
                            )
                            if it < 2:
                                r2 = small.tile([1, 512], F32, tag="nw_r",
                                                bufs=2)
                                nc.vector.tensor_mul(r2, r, w)
                                r = r2
                            else:
                                nc.vector.tensor_mul(m_row, r, w)

                    def finish_chunk(y_ps, h, ic):
                        # Unnormalized y straight to SBUF (softmax denominator
                        # cancels in subln); row-sum of squares, then rstd
                        # computed inline so the tail only broadcasts.
                        yslice = yfin[h][:, ic * 512:(ic + 1) * 512]
                        nc.scalar.copy(yslice, y_ps)
                        ysq = small.tile([128, 512], F32R, tag="ysq", bufs=1)
                        nc.vector.tensor_mul(
                            ysq, yslice.bitcast(F32), yslice.bitcast(F32)
                        )
                        ysum_ps = psum.tile([1, 512], F32, tag="ysum_ps", bufs=1)
                        nc.tensor.matmul(ysum_ps, onesc_t, ysq,
                                         start=True, stop=True)
                        nc.vector.tensor_copy(ysum_sb[h * TC + ic], ysum_ps)
                        rsqrt_inplace(ysum_sb[h * TC + ic])

                    def emit_qk(st, jt):
                        h, ic = st["h"], st["ic"]
                        rk = rk_all[:, h * 16 + jt:h * 16 + jt + 1]
                        s_ps = psum.tile([128, 512], F32, tag="s_ps", bufs=3)
                        pt = ppool.tile([128, 512], F32R, tag="pt")
                        i_lo = max(0, jt - 4 * ic)
                        lo = i_lo * 128
                        m = 4 * ic - jt
                        mi = m + 3  # index into bias tables
                        # columns beyond the ALiBi-relevant band contribute
                        # < e^-12 relative; worst (flattest) slope per
                        # in-group head across the 4 groups: 2^-(2+2h)
                        if h < 2:
                            smin = 2.0 ** (-(2 + 2 * h))
                            end = int(28.0 / smin) - 128 * m + 127
                            end = min(512, max(lo + 128, (end + 127) // 128 * 128))
                        else:
                            end = 512
                        nc.tensor.matmul(
                            s_ps[:, lo:end],
                            k_sb[h][:, jt * 128:(jt + 1) * 128],
                            st["qhat"][:, lo:end], start=True, stop=True,
                        )
                        if h < 2:
                            ramp = cpk[:, CPK_RAMP + 512 * h:
                                       CPK_RAMP + 512 * (h + 1)]
                            nc.vector.scalar_tensor_tensor(
                                s_ps[:, lo:end], s_ps[:, lo:end], rk,
                                ramp[:, lo:end], ALU.mult, ALU.add,
                            )
                            if jt >= 4 * ic:
                                src_b = s_ps[:, lo:lo + 128]
                                nc.vector.tensor_add(src_b, src_b, mask_t)
                            bias = cpk[:, CPK_MB + h * 16 + mi:
                                       CPK_MB + h * 16 + mi + 1]
                            nc.scalar.activation(
                                pt[:, lo:end], s_ps[:, lo:end], AF.Exp,
                                scale=1.0, bias=bias,
                            )
                        else:
                            if jt >= 4 * ic:
                                src_b = s_ps[:, lo:lo + 128]
                                nc.vector.tensor_add(src_b, src_b, mask_t)
                            bias = cpk[:, CPK_BT + (h - 2) * 16 + mi:
                                       CPK_BT + (h - 2) * 16 + mi + 1]
                            nc.scalar.activation(
                                pt[:, lo:end], s_ps[:, lo:end], AF.Exp,
                                scale=rk, bias=bias,
                            )
                        if i_lo > 0:
                            nc.gpsimd.tensor_copy(
                                pt[:, 0:lo], zeros_r[:, 0:lo]
                            )
                        if end < 512:
                            nc.gpsimd.tensor_copy(
                                pt[:, end:], zeros_r[:, 0:512 - end]
                            )
                        st["pt"][jt] = pt
                        st["av_lhs"][jt] = v_sb[jt][:, h * 128:(h + 1) * 128]

                    def emit_av(st, jt):
                        nc.tensor.matmul(
                            st["y_ps"], st["av_lhs"][jt], st["pt"][jt],
                            start=(jt == st["jt_lo"]),
                            stop=(jt == 4 * st["ic"] + 3),
                            skip_group_check=True,
                        )

                    pending = None
                    for h in range(HPG):
                        for ic in range(TC):
                            jt_lo = max(0, 4 * ic - M_THR[h])
                            st = dict(
                                h=h, ic=ic, jt_lo=jt_lo,
                                qhat=q_sb[h][:, ic * 512:(ic + 1) * 512],
                                y_ps=psum.tile([128, 512], F32, tag="big_ps",
                                               bufs=2, name="y_ps"),
                                pt={}, av_lhs={},
                            )
                            first = True
                            for jt in range(jt_lo, 4 * ic + 4):
                                emit_qk(st, jt)
                                if first and pending is not None:
                                    finish_chunk(pending["y_ps"], pending["h"],
                                                 pending["ic"])
                                first = False
                                if jt > jt_lo:
                                    emit_av(st, jt - 1)
                            emit_av(st, 4 * ic + 3)
                            pending = st
                    finish_chunk(pending["y_ps"], pending["h"], pending["ic"])

                    # batched subln rstd via vector-engine Newton rsqrt (no
                    # scalar Sqrt -> no Exp-table thrash), interleaved with
                    # the output projection. The priority bump keeps the
                    # tail's vector/scalar copies out of the attention
                    # window (attention is vector/scalar-paced).
                    tc.cur_priority += 1_000_000

                    def emit_subln(ic):
                        for h in range(HPG):
                            ysb_ps = psum.tile([128, 512], F32, tag="ps_b",
                                               bufs=2)
                            nc.tensor.matmul(
                                ysb_ps, onesr_t, ysum_sb[h * TC + ic],
                                start=True, stop=True,
                            )
                            yslice = yfin[h][:, ic * 512:(ic + 1) * 512]
                            nc.vector.tensor_mul(
                                yslice, yslice.bitcast(F32), ysb_ps
                            )

                    def emit_proj(ic, opool):
                        for tt in range(4 * ic, 4 * ic + 4):
                            for cc2 in range(2):
                                ot = opool.tile([128, 1024], F32, tag="ot")
                                for half in range(2):
                                    cc = 2 * cc2 + half
                                    ps = psum.tile([128, 512], F32,
                                                   tag="big_ps", bufs=2)
                                    for h in range(HPG):
                                        nc.tensor.matmul(
                                            ps,
                                            yfin[h][:, tt * 128:(tt + 1) * 128],
                                            wp_t[:, h, cc * 512:(cc + 1) * 512],
                                            start=(h == 0),
                                            stop=(h == HPG - 1),
                                        )
                                    dst = ot[:, half * 512:(half + 1) * 512]
                                    if (tt + cc2) % 2 == 0:
                                        nc.vector.tensor_copy(dst, ps)
                                    else:
                                        nc.scalar.copy(dst, ps)
                                nc.sync.dma_start(
                                    out=out[tt * 128:(tt + 1) * 128,
                                            cc2 * 1024:(cc2 + 1) * 1024],
                                    in_=ot,
                                )

                    with (
                        tc.tile_pool(name="psum4", bufs=1,
                                     space="PSUM") as psum,
                        tc.tile_pool(name="opool", bufs=2) as opool,
                    ):
                        emit_subln(0)
                        for ic in range(TC):
                            if ic + 1 < TC:
                                emit_subln(ic + 1)
                            emit_proj(ic, opool)

    nc.compile()
    return nc


_NC_CACHE = None


def _get_nc():
    global _NC_CACHE
    if _NC_CACHE is None:
        _NC_CACHE = _build()
    return _NC_CACHE


def _pack_w(Wsel):
    # [C, 512] -> [128, ft*ct*128]: row p holds, ft-major then ct-major, the
    # weight block W[ct*128+p, ft*128:(ft+1)*128]
    w4 = Wsel.reshape(NCT, 128, 4, 128)          # [ct, p, ft, d]
    return np.ascontiguousarray(
        w4.transpose(1, 2, 0, 3).reshape(128, 4 * NCT * 128)
    )


def kernel_in_maps(x, Wq, Wk, Wv, Wproj, q_rms_w, k_rms_w, subln_w):
    slopes = _alibi_slopes(H)

    x = np.asarray(x, dtype=np.float32)
    Wq = np.asarray(Wq, dtype=np.float32)
    Wk = np.asarray(Wk, dtype=np.float32)
    Wv = np.asarray(Wv, dtype=np.float32)
    Wproj = np.asarray(Wproj, dtype=np.float32)
    q_rms_w = np.asarray(q_rms_w, dtype=np.float32)
    k_rms_w = np.asarray(k_rms_w, dtype=np.float32)
    subln_w = np.asarray(subln_w, dtype=np.float32)

    wqk = (q_rms_w * k_rms_w).reshape(128)
    cmask = np.where(
        np.arange(128)[:, None] <= np.arange(128)[None, :], 0.0, NEG
    ).astype(np.float32)
    dj = np.arange(128, dtype=np.float32)
    cols = np.arange(512, dtype=np.float32)

    in_maps = []
    for b in range(B):
        xt = np.ascontiguousarray(x[b].T)
        for g in range(HG):
            heads = [g + 4 * j for j in range(HPG)]  # strided: slopes shrink with j
            csel = np.concatenate(
                [np.arange(hh * D, (hh + 1) * D) for hh in heads]
            )
            # sqrt(128): subln rstd is computed as rsqrt(sum y^2), i.e.
            # missing the 1/mean factor sqrt(128) - folded here.
            wproj_s = np.ascontiguousarray(
                Wproj[csel, :] * np.tile(subln_w, HPG)[:, None]
                * np.float32(math.sqrt(128.0))
            )
            cpk = np.zeros((128, CPK), np.float32)
            cpk[:, CPK_MASK:CPK_MASK + 128] = cmask
            cpk[:, CPK_WQK] = wqk
            for j, hh in enumerate(heads):
                slope = slopes[hh]
                for m in range(-3, 13):
                    if j < 2:
                        cpk[:, CPK_MB + j * 16 + m + 3] = -128.0 * slope * m
                    else:
                        cpk[:, CPK_BT + (j - 2) * 16 + m + 3] = slope * (
                            dj - 128.0 * m
                        )
                if j < 2:
                    cpk[:, CPK_RAMP + 512 * j:CPK_RAMP + 512 * (j + 1)] = (
                        slope * (dj[:, None] - cols[None, :])
                    )
            in_maps.append({
                "xt": xt,
                "wq": _pack_w(Wq[:, csel]),
                "wk": _pack_w(Wk[:, csel]),
                "wv": np.ascontiguousarray(Wv[:, csel]),
                "wp": wproj_s,
                "cpack": cpk,
            })

    return in_maps


def gather(results):
    outs = [r["out"] for r in results]
    final = np.stack(
        [sum(outs[b * HG + 1:(b + 1) * HG], outs[b * HG]) for b in range(B)]
    )
    return final.astype(np.float32)


def kernel(x, Wq, Wk, Wv, Wproj, q_rms_w, k_rms_w, subln_w):
    from concourse.bass_utils import run_bass_kernel_spmd

    in_maps = kernel_in_maps(x, Wq, Wk, Wv, Wproj, q_rms_w, k_rms_w, subln_w)
    res = run_bass_kernel_spmd(_get_nc(), in_maps, core_ids=list(range(8)))
    return gather(res.results)


if __name__ == "__main__":
    rng = np.random.default_rng(0)
    ins = {
        "x": rng.standard_normal((B, T, C), dtype=np.float32),
        "Wq": rng.standard_normal((C, H * D), dtype=np.float32) / math.sqrt(C),
        "Wk": rng.standard_normal((C, H * D), dtype=np.float32) / math.sqrt(C),
        "Wv": rng.standard_normal((C, H * D), dtype=np.float32) / math.sqrt(C),
        "Wproj": rng.standard_normal((H * D, C), dtype=np.float32) * 0.001,
        "q_rms_w": np.ones(D, np.float32),
        "k_rms_w": np.ones(D, np.float32),
        "subln_w": np.ones(D, np.float32),
    }
    y = kernel(**ins)
    print("kernel output", y.shape, y.dtype, float(np.abs(y).mean()))


# revision 42
# speedup vs baseline: 1.1953x; 1.1953x over previous
"""Causal self-attention (ALiBi + QK-RMSNorm + subln) on 8 TRN2 NeuronCores.

Sharding: 8 cores = 2 batches x 4 head-groups (4 heads / 128 head-dim each).
Per core: QKV projection for its 512 features, attention for its 4 heads,
and a partial output projection (row slice of Wproj); host sums the 4
partials per batch.

v4 design:
- q/k/v stay SBUF-resident; no DRAM round trip (phase A = two passes over x:
  k+v, then q). wk/wq are host-packed so ft-major tiles are contiguous DMAs.
- The softmax denominator is never computed: subln (RMSNorm over head_dim)
  is scale-invariant per (t, head), so per-column factors - the softmax
  denominator and the per-column ALiBi term - cancel. All heads use exact
  per-j-tile biases: exp arg = rk*s + slope*(dj - 128*m) via the activation
  scale/bias operands (m = 4*ic - jt, one bias column per (h, m)); the two
  steep heads (h<2) additionally add the exact per-column ramp
  (slope*dj - slope*col) on the vector engine so unnormalized y stays in
  fp32 range. j-tiles whose entire contribution underflows are skipped.
- Attention is software-pipelined: QK(jt+1) is emitted before AV(jt) so the
  in-order tensor queue never stalls on the exp of tile jt.
- subln rstd = rsqrt(sum y^2) via Newton iteration on the vector engine (no
  scalar Sqrt -> no activation-table thrash with Exp); the sqrt(128) and
  subln_w factors are folded into Wproj host-side.

All matmuls run as float32r (full-rate fp32 streaming with ~11-bit input
rounding, fp32 accumulation).
"""
import math

import numpy as np

import concourse.bacc as bacc
import concourse.bass as bass
import concourse.mybir as mybir
from concourse.tile import TileContext

F32 = mybir.dt.float32
F32R = mybir.dt.float32r
I32 = mybir.dt.int32
AF = mybir.ActivationFunctionType
ALU = mybir.AluOpType

B, T, C = 2, 2048, 2048
H, D = 16, 128
HG = 4          # head groups = cores per batch
HPG = 4         # heads per group
F = HPG * D     # 512 per-core qkv features
EPS = 1e-5
NEG = -1.0e30
TC = 4          # 512-wide t-chunks
NCT = 16        # 128-wide contraction tiles over C
MAGIC = 0x5F3759DF

# j-tile skip thresholds (worst case over the 4 groups' slopes per in-group
# head index): tiles with m = 4*ic - jt > M_THR contribute < e^-85 relative.
M_THR = {0: 1, 1: 4, 2: 99, 3: 99}

# packed-constants layout (columns of cpack [128, CPK])
CPK_MASK = 0          # 128: causal mask for the diagonal block
CPK_WQK = 128         # 1: q_rms_w * k_rms_w
CPK_RAMP = 129        # 2*512: per-column ramp slope*(dj - col) for h=0,1
CPK_MB = 1153         # 2*16: -128*slope*m for h=0,1 (m+3 indexed)
CPK_BT = 1185         # 2*16: slope*(dj - 128*m) for h=2,3 (m+3 indexed)
CPK = 1217


def _alibi_slopes(n_heads):
    def pow2(n):
        start = 2 ** (-(2 ** (-(math.log2(n) - 3))))
        return [start * start**i for i in range(n)]

    if math.log2(n_heads).is_integer():
        return pow2(n_heads)
    c = 2 ** math.floor(math.log2(n_heads))
    s = pow2(c)
    extra = _alibi_slopes(2 * c)
    return s + extra[0::2][: n_heads - c]


def _build():
    nc = bacc.Bacc("TRN2", target_bir_lowering=False)

    xt = nc.dram_tensor("xt", [C, T], F32R, kind="ExternalInput")
    # wq/wk host-packed: [128, ft*ct*128] so ft-major tiles are contiguous
    wq = nc.dram_tensor("wq", [128, 4 * NCT * 128], F32R, kind="ExternalInput")
    wk = nc.dram_tensor("wk", [128, 4 * NCT * 128], F32R, kind="ExternalInput")
    wv = nc.dram_tensor("wv", [C, F], F32R, kind="ExternalInput")
    wp = nc.dram_tensor("wp", [F, C], F32R, kind="ExternalInput")
    cpack = nc.dram_tensor("cpack", [128, CPK], F32, kind="ExternalInput")
    out = nc.dram_tensor("out", [T, C], F32, kind="ExternalOutput")

    xt_r = xt.rearrange("(ct p) t -> p ct t", p=128)
    wq_r = wq.rearrange("p (ft ct d) -> p ft ct d", ft=4, ct=NCT, d=128)
    wk_r = wk.rearrange("p (ft ct d) -> p ft ct d", ft=4, ct=NCT, d=128)
    wv_r = wv.rearrange("(ct p) f -> p ct f", p=128)
    wp_r = wp.rearrange("(ht p) c -> p ht c", p=128)

    with nc.allow_low_precision(reason="f32r rounding of matmul operands is intentional"), TileContext(nc) as tc:
        with (
            tc.tile_pool(name="consts", bufs=1) as consts,
            tc.tile_pool(name="kv", bufs=1) as kv,
        ):
            cpk = consts.tile([128, CPK], F32, tag="cpk")
            nc.sync.dma_start(out=cpk, in_=cpack[:, :])
            mask_t = cpk[:, CPK_MASK:CPK_MASK + 128]
            wqk_t = cpk[:, CPK_WQK:CPK_WQK + 1]

            onesc_t = consts.tile([128, 1], F32R, tag="onesc_t")
            nc.vector.memset(onesc_t.bitcast(F32), 1.0)
            onesr_t = consts.tile([1, 128], F32R, tag="onesr_t")
            nc.vector.memset(onesr_t.bitcast(F32), 1.0)
            eps_c = consts.tile([128, 1], F32, tag="eps_c")
            nc.vector.memset(eps_c, EPS)
            eps128_r = consts.tile([1, 1], F32, tag="eps128_r")
            nc.vector.memset(eps128_r, 128.0 * EPS)
            magic_i = consts.tile([1, 512], I32, tag="magic_i")
            nc.vector.memset(magic_i, MAGIC)
            zeros_r = consts.tile([128, 384], F32R, tag="zeros_r")
            nc.vector.memset(zeros_r.bitcast(F32), 0.0)
            ones_m = consts.tile([128, 256], F32R, tag="ones_m")
            nc.vector.memset(ones_m.bitcast(F32), 1.0)

            # rk columns (1/rms of k) for all (head, j-tile)
            rk_all = consts.tile([128, HPG * 16], F32, tag="rk_all")

            # SBUF-resident k-hat (per head, [d, t]) and v (per t-block, [t, f])
            k_sb = [kv.tile([128, T], F32R, tag=f"k_sb{h}", name=f"k_sb{h}")
                    for h in range(HPG)]
            v_sb = [kv.tile([128, F], F32R, tag=f"v_sb{j}", name=f"v_sb{j}")
                    for j in range(16)]

            # ---------------- Phase A pass 1: k + v ----------------------
            with (
                tc.tile_pool(name="w1", bufs=1) as w1,
                tc.tile_pool(name="xp1", bufs=1) as xp1,
                tc.tile_pool(name="stg1", bufs=1) as stg1,
                tc.tile_pool(name="psum1", bufs=1, space="PSUM") as psum,
            ):
                wk_f = []
                for ftg in range(4):
                    t_ = w1.tile([128, NCT, 128], F32R, tag=f"wk{ftg}",
                                 name=f"wk{ftg}")
                    nc.sync.dma_start(out=t_, in_=wk_r[:, ftg])
                    wk_f.append(t_)
                wv_g = []
                for g in range(4):
                    t_ = w1.tile([128, 4, 512], F32R, tag=f"wv{g}", name=f"wv{g}")
                    nc.sync.dma_start(out=t_, in_=wv_r[:, 4 * g:4 * g + 4, :])
                    wv_g.append(t_)

                # keep the PE busy while the first DMAs land so HAM
                # un-throttles before the real chains start
                warm_ps = psum.tile([1, 256], F32, tag="warm", bufs=1,
                                    name="warm_ps")
                for _ in range(30):
                    nc.tensor.matmul(warm_ps, onesc_t, ones_m,
                                     start=True, stop=True)

                pending_aux = []
                for tch in range(TC):
                    xg = []
                    for s in range(4):
                        t_ = xp1.tile([128, 4, 512], F32R, tag="x1", bufs=6,
                                      name="x1t")
                        nc.scalar.dma_start(
                            out=t_,
                            in_=xt_r[:, 4 * s:4 * s + 4,
                                     tch * 512:(tch + 1) * 512],
                        )
                        xg.append(t_)

                    ksq = []
                    for ft in range(4):
                        ps = psum.tile([128, 512], F32, tag="big_ps", bufs=3)
                        for ct in range(NCT):
                            nc.tensor.matmul(
                                ps,
                                wk_f[ft][:, ct, :],
                                xg[ct // 4][:, ct % 4, :],
                                start=(ct == 0),
                                stop=(ct == NCT - 1),
                            )
                        st = stg1.tile([128, 512], F32R, tag="st", bufs=2)
                        nc.scalar.copy(st, ps)
                        kq = stg1.tile([128, 512], F32R, tag=f"ksq{ft}", bufs=2)
                        nc.vector.tensor_mul(kq, st.bitcast(F32), st.bitcast(F32))
                        ksq.append(kq)
                        nc.vector.tensor_scalar_mul(
                            k_sb[ft][:, tch * 512:(tch + 1) * 512],
                            st.bitcast(F32), scalar1=wqk_t,
                        )
                    for ts4 in range(4):
                        ps = psum.tile([128, 512], F32, tag="big_ps", bufs=3)
                        for ct in range(NCT):
                            nc.tensor.matmul(
                                ps,
                                xg[ct // 4][:, ct % 4, ts4 * 128:(ts4 + 1) * 128],
                                wv_g[ct // 4][:, ct % 4, :],
                                start=(ct == 0),
                                stop=(ct == NCT - 1),
                            )
                        nc.scalar.copy(v_sb[tch * 4 + ts4], ps)

                    # k-norm aux for this tch, emitted one tch later so the
                    # small matmuls never stall the tensor queue.
                    def k_aux(tch=tch, ksq=ksq):
                        for ft in range(4):
                            for ts4 in range(4):
                                jt = tch * 4 + ts4
                                psk = psum.tile([128, 256], F32, tag="psk",
                                                bufs=2)
                                nc.tensor.matmul(
                                    psk, ksq[ft][:, ts4 * 128:(ts4 + 1) * 128],
                                    ones_m, start=True, stop=True,
                                )
                                col = rk_all[:, ft * 16 + jt:ft * 16 + jt + 1]
                                nc.scalar.activation(
                                    col, psk[:, 0:1], AF.Sqrt,
                                    scale=1.0 / 128.0, bias=eps_c,
                                )
                                nc.vector.reciprocal(col, col)

                    if pending_aux:
                        pending_aux.pop()()
                    pending_aux.append(k_aux)
                for fn in pending_aux:
                    fn()

            # ---------------- Phase A pass 2: q -------------------------
            with tc.tile_pool(name="qpool", bufs=1) as qpool:
                q_sb = [qpool.tile([128, T], F32R, tag=f"q_sb{h}", name=f"q_sb{h}")
                        for h in range(HPG)]
                with (
                    tc.tile_pool(name="w2", bufs=1) as w2,
                    tc.tile_pool(name="xp2", bufs=1) as xp2,
                    tc.tile_pool(name="stg2", bufs=1) as stg2,
                    tc.tile_pool(name="psum2", bufs=1, space="PSUM") as psum,
                ):
                    wq_f = []
                    for ftg in range(4):
                        t_ = w2.tile([128, NCT, 128], F32R, tag=f"wq{ftg}",
                                     name=f"wq{ftg}")
                        nc.sync.dma_start(out=t_, in_=wq_r[:, ftg])
                        wq_f.append(t_)

                    pending_aux = []
                    for tch in range(TC):
                        xg = []
                        for s in range(4):
                            t_ = xp2.tile([128, 4, 512], F32R, tag="x2", bufs=6,
                                          name="x2t")
                            nc.scalar.dma_start(
                                out=t_,
                                in_=xt_r[:, 4 * s:4 * s + 4,
                                         tch * 512:(tch + 1) * 512],
                            )
                            xg.append(t_)

                        for ft in range(4):
                            ps = psum.tile([128, 512], F32, tag="big_ps", bufs=3)
                            for ct in range(NCT):
                                nc.tensor.matmul(
                                    ps,
                                    wq_f[ft][:, ct, :],
                                    xg[ct // 4][:, ct % 4, :],
                                    start=(ct == 0),
                                    stop=(ct == NCT - 1),
                                )
                            st = stg2.tile([128, 512], F32R, tag="st", bufs=4)
                            nc.scalar.copy(st, ps)
                            qsq = stg2.tile([128, 512], F32R, tag="qsq", bufs=2)
                            nc.vector.tensor_mul(
                                qsq, st.bitcast(F32), st.bitcast(F32)
                            )

                            def q_aux(tch=tch, ft=ft, st=st, qsq=qsq):
                                ps_row = psum.tile([1, 512], F32, tag="ps_row",
                                                   bufs=2)
                                nc.tensor.matmul(ps_row, onesc_t, qsq,
                                                 start=True, stop=True)
                                rq_f = stg2.tile([1, 512], F32, tag="rq_f")
                                nc.scalar.activation(
                                    rq_f, ps_row, AF.Sqrt, scale=1.0,
                                    bias=eps128_r,
                                )
                                rq_f2 = stg2.tile([1, 512], F32, tag="rq_f2")
                                nc.vector.reciprocal_approx_fast(rq_f2, rq_f)
                                rq_row = stg2.tile([1, 512], F32R, tag="rq_row")
                                nc.vector.tensor_copy(rq_row, rq_f2)
                                ps_b = psum.tile([128, 512], F32, tag="ps_b",
                                                 bufs=2)
                                nc.tensor.matmul(ps_b, onesr_t, rq_row,
                                                 start=True, stop=True)
                                nc.vector.tensor_mul(
                                    q_sb[ft][:, tch * 512:(tch + 1) * 512],
                                    st.bitcast(F32), ps_b,
                                )

                            if pending_aux:
                                pending_aux.pop()()
                            pending_aux.append(q_aux)
                    for fn in pending_aux:
                        fn()

                # ---------------- Phase B: attention per head -----------
                with (
                    tc.tile_pool(name="head", bufs=1) as head,
                    tc.tile_pool(name="ppool", bufs=3) as ppool,
                    tc.tile_pool(name="yfin_pool", bufs=1) as yfin_pool,
                    tc.tile_pool(name="small", bufs=2) as small,
                ):
                    wp_t = head.tile([128, HPG, C], F32R, tag="wp_t", bufs=1)
                    nc.sync.dma_start(out=wp_t, in_=wp_r)

                    # yfin[0] is fresh; heads 1-3 reuse k_sb[h-1], which is
                    # fully consumed before head h writes it.
                    yfin = [yfin_pool.tile([128, T], F32R, tag="yfin0",
                                           name="yfin0")]
                    for h in range(1, HPG):
                        yfin.append(k_sb[h - 1])

                    ysum_sb = []  # [1,512] per (h, ic) for deferred subln rstd
                    for idx in range(HPG * TC):
                        yt = small.tile([1, 512], F32R, tag=f"ysum{idx}", bufs=1,
                                        name=f"ysum{idx}")
                        ysum_sb.append(yt)

                    def rsqrt_inplace(m_row):
                        # m_row <- rsqrt(m_row), Newton iteration on the
                        # vector engine (no scalar Sqrt -> no table thrash)
                        half = small.tile([1, 512], I32, tag="nw_h", bufs=1)
                        nc.vector.tensor_scalar(
                            half, m_row.bitcast(I32), scalar1=1, scalar2=None,
                            op0=ALU.logical_shift_right,
                        )
                        r = small.tile([1, 512], F32, tag="nw_r", bufs=2)
                        nc.vector.tensor_sub(r.bitcast(I32), magic_i, half)
                        for it in range(3):
                            u = small.tile([1, 512], F32, tag="nw_t", bufs=1)
                            nc.vector.tensor_mul(u, r, r)
                            v = small.tile([1, 512], F32, tag="nw_v", bufs=1)
                            nc.vector.tensor_mul(v, m_row.bitcast(F32), u)
                            w = small.tile([1, 512], F32, tag="nw_t", bufs=1)
                            nc.vector.tensor_scalar(
                                w, v, scalar1=-0.5, scalar2=1.5,
                                op0=ALU.mult, op1=ALU.add,
                            )
                            if it < 2:
                                r2 = small.tile([1, 512], F32, tag="nw_r",
                                                bufs=2)
                                nc.vector.tensor_mul(r2, r, w)
                                r = r2
                            else:
                                nc.vector.tensor_mul(m_row, r, w)

                    def finish_chunk(y_ps, h, ic):
                        # Unnormalized y straight to SBUF (softmax denominator
                        # cancels in subln); row-sum of squares, then rstd
                        # computed inline so the tail only broadcasts.
                        yslice = yfin[h][:, ic * 512:(ic + 1) * 512]
                        nc.scalar.copy(yslice, y_ps)
                        ysq = small.tile([128, 512], F32R, tag="ysq", bufs=1)
                        nc.vector.tensor_mul(
                            ysq, yslice.bitcast(F32), yslice.bitcast(F32)
                        )
                        ysum_ps = psum.tile([1, 512], F32, tag="ysum_ps", bufs=1)
                        nc.tensor.matmul(ysum_ps, onesc_t, ysq,
                                         start=True, stop=True)
                        nc.vector.tensor_copy(ysum_sb[h * TC + ic], ysum_ps)
                        rsqrt_inplace(ysum_sb[h * TC + ic])

                    def emit_qk(st, jt):
                        h, ic = st["h"], st["ic"]
                        rk = rk_all[:, h * 16 + jt:h * 16 + jt + 1]
                        s_ps = psum.tile([128, 512], F32, tag="s_ps", bufs=5)
                        pt = ppool.tile([128, 512], F32R, tag="pt")
                        i_lo = max(0, jt - 4 * ic)
                        lo = i_lo * 128
                        m = 4 * ic - jt
                        mi = m + 3  # index into bias tables
                        # columns beyond the ALiBi-relevant band contribute
                        # < e^-12 relative; worst (flattest) slope per
                        # in-group head across the 4 groups: 2^-(2+2h)
                        if h < 2:
                            smin = 2.0 ** (-(2 + 2 * h))
                            end = int(28.0 / smin) - 128 * m + 127
                            end = min(512, max(lo + 128, (end + 127) // 128 * 128))
                        else:
                            end = 512
                        nc.tensor.matmul(
                            s_ps[:, lo:end],
                            k_sb[h][:, jt * 128:(jt + 1) * 128],
                            st["qhat"][:, lo:end], start=True, stop=True,
                        )
                        if h < 2:
                            ramp = cpk[:, CPK_RAMP + 512 * h:
                                       CPK_RAMP + 512 * (h + 1)]
                            nc.vector.scalar_tensor_tensor(
                                s_ps[:, lo:end], s_ps[:, lo:end], rk,
                                ramp[:, lo:end], ALU.mult, ALU.add,
                            )
                            if jt >= 4 * ic:
                                src_b = s_ps[:, lo:lo + 128]
                                nc.vector.tensor_add(src_b, src_b, mask_t)
                            bias = cpk[:, CPK_MB + h * 16 + mi:
                                       CPK_MB + h * 16 + mi + 1]
                            nc.scalar.activation(
                                pt[:, lo:end], s_ps[:, lo:end], AF.Exp,
                                scale=1.0, bias=bias,
                            )
                        else:
                            if jt >= 4 * ic:
                                src_b = s_ps[:, lo:lo + 128]
                                nc.vector.tensor_add(src_b, src_b, mask_t)
                            bias = cpk[:, CPK_BT + (h - 2) * 16 + mi:
                                       CPK_BT + (h - 2) * 16 + mi + 1]
                            nc.scalar.activation(
                                pt[:, lo:end], s_ps[:, lo:end], AF.Exp,
                                scale=rk, bias=bias,
                            )
                        if i_lo > 0:
                            nc.gpsimd.tensor_copy(
                                pt[:, 0:lo], zeros_r[:, 0:lo]
                            )
                        if end < 512:
                            nc.gpsimd.tensor_copy(
                                pt[:, end:], zeros_r[:, 0:512 - end]
                            )
                        st["pt"][jt] = pt
                        st["av_lhs"][jt] = v_sb[jt][:, h * 128:(h + 1) * 128]

                    def emit_av(st, jt):
                        nc.tensor.matmul(
                            st["y_ps"], st["av_lhs"][jt], st["pt"][jt],
                            start=(jt == st["jt_lo"]),
                            stop=(jt == 4 * st["ic"] + 3),
                            skip_group_check=True,
                        )

                    pending = None
                    for h in range(HPG):
                        for ic in range(TC):
                            jt_lo = max(0, 4 * ic - M_THR[h])
                            st = dict(
                                h=h, ic=ic, jt_lo=jt_lo,
                                qhat=q_sb[h][:, ic * 512:(ic + 1) * 512],
                                y_ps=psum.tile([128, 512], F32, tag="big_ps",
                                               bufs=2, name="y_ps"),
                                pt={}, av_lhs={},
                            )
                            first = True
                            for jt in range(jt_lo, 4 * ic + 4):
                                emit_qk(st, jt)
                                if first and pending is not None:
                                    finish_chunk(pending["y_ps"], pending["h"],
                                                 pending["ic"])
                                first = False
                                if jt > jt_lo:
                                    emit_av(st, jt - 1)
                            emit_av(st, 4 * ic + 3)
                            pending = st
                    finish_chunk(pending["y_ps"], pending["h"], pending["ic"])

                    # batched subln rstd via vector-engine Newton rsqrt (no
                    # scalar Sqrt -> no Exp-table thrash), interleaved with
                    # the output projection. The priority bump keeps the
                    # tail's vector/scalar copies out of the attention
                    # window (attention is vector/scalar-paced).
                    tc.cur_priority += 1_000_000

                    def emit_subln(ic):
                        for h in range(HPG):
                            ysb_ps = psum.tile([128, 512], F32, tag="ps_b",
                                               bufs=2)
                            nc.tensor.matmul(
                                ysb_ps, onesr_t, ysum_sb[h * TC + ic],
                                start=True, stop=True,
                            )
                            yslice = yfin[h][:, ic * 512:(ic + 1) * 512]
                            nc.vector.tensor_mul(
                                yslice, yslice.bitcast(F32), ysb_ps
                            )

                    def emit_proj(ic, opool):
                        for tt in range(4 * ic, 4 * ic + 4):
                            for cc2 in range(2):
                                ot = opool.tile([128, 1024], F32, tag="ot")
                                for half in range(2):
                                    cc = 2 * cc2 + half
                                    ps = psum.tile([128, 512], F32,
                                                   tag="big_ps", bufs=2)
                                    for h in range(HPG):
                                        nc.tensor.matmul(
                                            ps,
                                            yfin[h][:, tt * 128:(tt + 1) * 128],
                                            wp_t[:, h, cc * 512:(cc + 1) * 512],
                                            start=(h == 0),
                                            stop=(h == HPG - 1),
                                        )
                                    dst = ot[:, half * 512:(half + 1) * 512]
                                    if (tt + cc2) % 2 == 0:
                                        nc.vector.tensor_copy(dst, ps)
                                    else:
                                        nc.scalar.copy(dst, ps)
                                nc.sync.dma_start(
                                    out=out[tt * 128:(tt + 1) * 128,
                                            cc2 * 1024:(cc2 + 1) * 1024],
                                    in_=ot,
                                )

                    with (
                        tc.tile_pool(name="psum4", bufs=1,
                                     space="PSUM") as psum,
                        tc.tile_pool(name="opool", bufs=2) as opool,
                    ):
                        emit_subln(0)
                        for ic in range(TC):
                            if ic + 1 < TC:
                                emit_subln(ic + 1)
                            emit_proj(ic, opool)

    nc.compile()
    return nc


_NC_CACHE = None


def _get_nc():
    global _NC_CACHE
    if _NC_CACHE is None:
        _NC_CACHE = _build()
    return _NC_CACHE


def _pack_w(Wsel):
    # [C, 512] -> [128, ft*ct*128]: row p holds, ft-major then ct-major, the
    # weight block W[ct*128+p, ft*128:(ft+1)*128]
    w4 = Wsel.reshape(NCT, 128, 4, 128)          # [ct, p, ft, d]
    return np.ascontiguousarray(
        w4.transpose(1, 2, 0, 3).reshape(128, 4 * NCT * 128)
    )


def kernel_in_maps(x, Wq, Wk, Wv, Wproj, q_rms_w, k_rms_w, subln_w):
    slopes = _alibi_slopes(H)

    x = np.asarray(x, dtype=np.float32)
    Wq = np.asarray(Wq, dtype=np.float32)
    Wk = np.asarray(Wk, dtype=np.float32)
    Wv = np.asarray(Wv, dtype=np.float32)
    Wproj = np.asarray(Wproj, dtype=np.float32)
    q_rms_w = np.asarray(q_rms_w, dtype=np.float32)
    k_rms_w = np.asarray(k_rms_w, dtype=np.float32)
    subln_w = np.asarray(subln_w, dtype=np.float32)

    wqk = (q_rms_w * k_rms_w).reshape(128)
    cmask = np.where(
        np.arange(128)[:, None] <= np.arange(128)[None, :], 0.0, NEG
    ).astype(np.float32)
    dj = np.arange(128, dtype=np.float32)
    cols = np.arange(512, dtype=np.float32)

    in_maps = []
    for b in range(B):
        xt = np.ascontiguousarray(x[b].T)
        for g in range(HG):
            heads = [g + 4 * j for j in range(HPG)]  # strided: slopes shrink with j
            csel = np.concatenate(
                [np.arange(hh * D, (hh + 1) * D) for hh in heads]
            )
            # sqrt(128): subln rstd is computed as rsqrt(sum y^2), i.e.
            # missing the 1/mean factor sqrt(128) - folded here.
            wproj_s = np.ascontiguousarray(
                Wproj[csel, :] * np.tile(subln_w, HPG)[:, None]
                * np.float32(math.sqrt(128.0))
            )
            cpk = np.zeros((128, CPK), np.float32)
            cpk[:, CPK_MASK:CPK_MASK + 128] = cmask
            cpk[:, CPK_WQK] = wqk
            for j, hh in enumerate(heads):
                slope = slopes[hh]
                for m in range(-3, 13):
                    if j < 2:
                        cpk[:, CPK_MB + j * 16 + m + 3] = -128.0 * slope * m
                    else:
                        cpk[:, CPK_BT + (j - 2) * 16 + m + 3] = slope * (
                            dj - 128.0 * m
                        )
                if j < 2:
                    cpk[:, CPK_RAMP + 512 * j:CPK_RAMP + 512 * (j + 1)] = (
                        slope * (dj[:, None] - cols[None, :])
                    )
            in_maps.append({
                "xt": xt,
                "wq": _pack_w(Wq[:, csel]),
                "wk": _pack_w(Wk[:, csel]),
                "wv": np.ascontiguousarray(Wv[:, csel]),
                "wp": wproj_s,
                "cpack": cpk,
            })

    return in_maps


def gather(results):
    outs = [r["out"] for r in results]
    final = np.stack(
        [sum(outs[b * HG + 1:(b + 1) * HG], outs[b * HG]) for b in range(B)]
    )
    return final.astype(np.float32)


def kernel(x, Wq, Wk, Wv, Wproj, q_rms_w, k_rms_w, subln_w):
    from concourse.bass_utils import run_bass_kernel_spmd

    in_maps = kernel_in_maps(x, Wq, Wk, Wv, Wproj, q_rms_w, k_rms_w, subln_w)
    res = run_bass_kernel_spmd(_get_nc(), in_maps, core_ids=list(range(8)))
    return gather(res.results)


if __name__ == "__main__":
    rng = np.random.default_rng(0)
    ins = {
        "x": rng.standard_normal((B, T, C), dtype=np.float32),
        "Wq": rng.standard_normal((C, H * D), dtype=np.float32) / math.sqrt(C),
        "Wk": rng.standard_normal((C, H * D), dtype=np.float32) / math.sqrt(C),
        "Wv": rng.standard_normal((C, H * D), dtype=np.float32) / math.sqrt(C),
        "Wproj": rng.standard_normal((H * D, C), dtype=np.float32) * 0.001,
        "q_rms_w": np.ones(D, np.float32),
        "k_rms_w": np.ones(D, np.float32),
        "subln_w": np.ones(D, np.float32),
    }
    y = kernel(**ins)
    print("kernel output", y.shape, y.dtype, float(np.abs(y).mean()))
